# revision 3
# baseline (speedup 1.0000x reference)
"""GraphSAGE encoder (2x SAGEConv, mean aggregation) on 8 trn2 NeuronCores.

Strategy (edge sharding by destination-node range == node sharding):
  - Host: sort edges by dst, partition into 8 contiguous dst ranges
    (12544 nodes per core). Within a core, edges are grouped per
    128-node "tile" and padded to 128-edge blocks (shared per-tile
    block schedule across cores so one SPMD program serves all 8).
  - Device, per layer: for each tile, indirect-DMA gather feat[src]
    for each 128-edge block, scale rows by 1/deg(dst) (ScalarE), build
    a one-hot (dst == node) matrix (VectorE), and accumulate
    feat_scaled.T @ onehot into PSUM (TensorE) -> meanT [64, 128].
    Epilogue: meanT @ Wl + selfT @ Wr + b (TensorE) -> relu (ScalarE),
    transpose back to row-major (TensorE) and store.
  - AllGather of h between the two layers (layer-2 gathers need all
    nodes' h); weights replicated.
"""

import os
import sys
import types

import numpy as np

N = 100000
E = 1600000
D = 64
NC = 8
NPAD = 100352
S = NPAD // NC  # 12544 nodes per core
TPC = S // 128  # 98 tiles per core
P = 128


def _install_ntff_shim():
    """Provide antenv.axon_hooks (NTFF profile hook) + disable artifact
    upload, so trace=True works in this container. No-op if unavailable."""
    try:
        import antenv

        if "antenv.axon_hooks" not in sys.modules:
            mod = types.ModuleType("antenv.axon_hooks")
            mod._hook = None
            mod.set_axon_ntff_profile_hook = lambda h: setattr(mod, "_hook", h)
            mod.get_axon_ntff_profile_hook = lambda: mod._hook
            sys.modules["antenv.axon_hooks"] = mod
            antenv.axon_hooks = mod
        from trn_agent_boot.trn_boot import _ntff_profile_via_ctypes

        hook = _ntff_profile_via_ctypes("/opt/axon/libaxon_pjrt.so")
        sys.modules["antenv.axon_hooks"].set_axon_ntff_profile_hook(hook)
        from concourse import bass_utils

        bass_utils.upload_artifacts = lambda tmpdir: "local://" + tmpdir
    except Exception:
        pass


def _split_multi_waits(nc):
    """walrus in this toolchain accepts at most ONE sync-wait per
    instruction; Tile attaches several. Split extras into standalone
    InstEventSemaphore waits on the same engine."""
    import concourse.mybir as mybir

    ctr = 0
    for bb in nc.main_func.blocks:
        out = []
        for ins in bb.instructions:
            si = ins.sync_info
            if si is not None and si.on_wait is not None and len(si.on_wait) > 1:
                waits = list(si.on_wait)
                for w in waits[:-1]:
                    ctr += 1
                    out.append(
                        mybir.InstEventSemaphore(
                            name=f"wsplit-{ctr}",
                            engine=ins.engine,
                            sync_info=mybir.SyncInfo(on_wait=[w], on_update=[]),
                        )
                    )
                ins.sync_info = mybir.SyncInfo(
                    on_wait=[waits[-1]], on_update=list(si.on_update or [])
                )
            out.append(ins)
        bb.instructions[:] = out


def _prep_edges(edge_index):
    """Sort by dst, partition by dst range, build per-core block arrays.

    Returns (nblk_t [TPC], src_a [NC,P,NBLK] i32, rel_a [NC,P,NBLK] f32,
    inv_a [NC,P,NBLK] f32)."""
    src = np.asarray(edge_index[0], dtype=np.int64)
    dst = np.asarray(edge_index[1], dtype=np.int64)
    order = np.argsort(dst, kind="stable")
    s_src = src[order].astype(np.int32)
    s_dst = dst[order].astype(np.int32)
    cnt = np.bincount(dst, minlength=N).astype(np.float32)
    inv = (1.0 / np.maximum(cnt, 1.0)).astype(np.float32)
    inv_e = inv[s_dst]

    n_tiles = NPAD // P  # 784 global tiles
    tile_id = s_dst >> 7
    counts = np.bincount(tile_id, minlength=n_tiles)
    ct = counts.reshape(NC, TPC)
    nblk_t = np.maximum(1, -(-ct // P)).max(axis=0).astype(np.int64)  # [TPC]
    NBLK = int(nblk_t.sum())

    src_a = np.zeros((NC, P, NBLK), np.int32)
    rel_a = np.full((NC, P, NBLK), -1.0, np.float32)
    inv_a = np.zeros((NC, P, NBLK), np.float32)

    tile_starts = np.zeros(n_tiles + 1, np.int64)
    np.cumsum(counts, out=tile_starts[1:])
    blk_starts = np.zeros(TPC + 1, np.int64)
    np.cumsum(nblk_t, out=blk_starts[1:])

    for c in range(NC):
        for t in range(TPC):
            gt = c * TPC + t
            e0, e1 = tile_starts[gt], tile_starts[gt + 1]
            ne = int(e1 - e0)
            if ne == 0:
                continue
            nb = -(-ne // P)
            padded = nb * P
            b0 = int(blk_starts[t])
            sseg = np.zeros(padded, np.int32)
            sseg[:ne] = s_src[e0:e1]
            rseg = np.full(padded, -1.0, np.float32)
            rseg[:ne] = (s_dst[e0:e1] - gt * P).astype(np.float32)
            iseg = np.zeros(padded, np.float32)
            iseg[:ne] = inv_e[e0:e1]
            src_a[c, :, b0 : b0 + nb] = sseg.reshape(nb, P).T
            rel_a[c, :, b0 : b0 + nb] = rseg.reshape(nb, P).T
            inv_a[c, :, b0 : b0 + nb] = iseg.reshape(nb, P).T

    return nblk_t, src_a, rel_a, inv_a


def _build_nc(nblk_t):
    import concourse.bass as bass
    import concourse.mybir as mybir
    import concourse.tile as tile
    from concourse.masks import make_identity

    NBLK = int(nblk_t.sum())
    f32 = mybir.dt.float32

    nc = bass.Bass(num_devices=NC)
    x = nc.dram_tensor("x", [NPAD, D], f32, kind="ExternalInput")
    xT = nc.dram_tensor("xT", [D, S], f32, kind="ExternalInput")
    srcs = nc.dram_tensor("srcs", [P, NBLK], mybir.dt.int32, kind="ExternalInput")
    rels = nc.dram_tensor("rels", [P, NBLK], f32, kind="ExternalInput")
    invs = nc.dram_tensor("invs", [P, NBLK], f32, kind="ExternalInput")
    iota = nc.dram_tensor("iota", [P, P], f32, kind="ExternalInput")
    w1l = nc.dram_tensor("w1l", [D, D], f32, kind="ExternalInput")
    w1r = nc.dram_tensor("w1r", [D, D], f32, kind="ExternalInput")
    w2l = nc.dram_tensor("w2l", [D, D], f32, kind="ExternalInput")
    w2r = nc.dram_tensor("w2r", [D, D], f32, kind="ExternalInput")
    b1 = nc.dram_tensor("b1", [D, 1], f32, kind="ExternalInput")
    b2 = nc.dram_tensor("b2", [D, 1], f32, kind="ExternalInput")
    out = nc.dram_tensor("out", [S, D], f32, kind="ExternalOutput")

    h_own = nc.dram_tensor("h_own", [S, D], f32, kind="Internal")
    h_full = nc.dram_tensor("h_full", [NPAD, D], f32, kind="Internal")

    with tile.TileContext(nc) as tc:
        with (
            tc.tile_pool(name="meta", bufs=1) as meta,
            tc.tile_pool(name="selfT", bufs=1) as selfp,
            tc.tile_pool(name="gat", bufs=12) as gat,
            tc.tile_pool(name="gsc", bufs=8) as gscp,
            tc.tile_pool(name="oh", bufs=8) as ohp,
            tc.tile_pool(name="acc", bufs=2, space="PSUM") as accp,
            tc.tile_pool(name="hps", bufs=2, space="PSUM") as hpsp,
            tc.tile_pool(name="trp", bufs=2, space="PSUM") as trpp,
            tc.tile_pool(name="epi", bufs=4) as epi,
        ):
            # --- constants / metadata, loaded once ---
            src_sb = meta.tile([P, NBLK], mybir.dt.int32)
            nc.sync.dma_start(out=src_sb[:], in_=srcs[:])
            rel_sb = meta.tile([P, NBLK], f32)
            nc.sync.dma_start(out=rel_sb[:], in_=rels[:])
            inv_sb = meta.tile([P, NBLK], f32)
            nc.sync.dma_start(out=inv_sb[:], in_=invs[:])
            iota_sb = meta.tile([P, P], f32)
            nc.sync.dma_start(out=iota_sb[:], in_=iota[:])
            ident_sb = meta.tile([P, P], f32)
            make_identity(nc, ident_sb[:])
            w1l_sb = meta.tile([D, D], f32)
            nc.sync.dma_start(out=w1l_sb[:], in_=w1l[:])
            w1r_sb = meta.tile([D, D], f32)
            nc.sync.dma_start(out=w1r_sb[:], in_=w1r[:])
            w2l_sb = meta.tile([D, D], f32)
            nc.sync.dma_start(out=w2l_sb[:], in_=w2l[:])
            w2r_sb = meta.tile([D, D], f32)
            nc.sync.dma_start(out=w2r_sb[:], in_=w2r[:])
            b1_sb = meta.tile([D, 1], f32)
            nc.sync.dma_start(out=b1_sb[:], in_=b1[:])
            b2_sb = meta.tile([D, 1], f32)
            nc.sync.dma_start(out=b2_sb[:], in_=b2[:])

            xT_sb = selfp.tile([D, S], f32)
            nc.sync.dma_start(out=xT_sb[:], in_=xT[:])
            hT_sb = selfp.tile([D, S], f32)

            def layer(feat_dram, self_sb, wl_sb, wr_sb, b_sb, relu, dst_dram):
                B = 0
                for t in range(TPC):
                    nb = int(nblk_t[t])
                    acc = accp.tile([D, P], f32, space="PSUM")
                    for j in range(nb):
                        g = gat.tile([P, D], f32)
                        nc.gpsimd.indirect_dma_start(
                            out=g[:],
                            out_offset=None,
                            in_=feat_dram[:],
                            in_offset=bass.IndirectOffsetOnAxis(
                                ap=src_sb[:, B : B + 1], axis=0
                            ),
                        )
                        gs = gscp.tile([P, D], f32)
                        nc.scalar.activation(
                            out=gs[:],
                            in_=g[:],
                            func=mybir.ActivationFunctionType.Copy,
                            bias=0.0,
                            scale=inv_sb[:, B : B + 1],
                        )
                        oh = ohp.tile([P, P], f32)
                        nc.vector.tensor_scalar(
                            out=oh[:],
                            in0=iota_sb[:],
                            scalar1=rel_sb[:, B : B + 1],
                            scalar2=None,
                            op0=mybir.AluOpType.is_equal,
                        )
                        nc.tensor.matmul(
                            out=acc[:],
                            lhsT=gs[:],
                            rhs=oh[:],
                            start=(j == 0),
                            stop=(j == nb - 1),
                        )
                        B += 1
                    # epilogue: acc [D,128] = meanT tile
                    meanT = epi.tile([D, P], f32, tag="meanT")
                    nc.vector.tensor_copy(out=meanT[:], in_=acc[:])
                    hps = hpsp.tile([D, P], f32, space="PSUM")
                    nc.tensor.matmul(
                        out=hps[:], lhsT=wl_sb[:], rhs=meanT[:], start=True, stop=False
                    )
                    nc.tensor.matmul(
                        out=hps[:],
                        lhsT=wr_sb[:],
                        rhs=self_sb[:, t * P : (t + 1) * P],
                        start=False,
                        stop=True,
                    )
                    # bias (+ relu) -> hT slice [D, 128] of dstT_sb
                    if relu:
                        nc.scalar.activation(
                            out=hT_sb[:, t * P : (t + 1) * P],
                            in_=hps[:],
                            func=mybir.ActivationFunctionType.Relu,
                            bias=b_sb[:, 0:1],
                            scale=1.0,
                        )
                        rowsrc = hT_sb[:, t * P : (t + 1) * P]
                    else:
                        outT = epi.tile([D, P], f32, tag="outT")
                        nc.vector.tensor_scalar(
                            out=outT[:],
                            in0=hps[:],
                            scalar1=b_sb[:, 0:1],
                            scalar2=None,
                            op0=mybir.AluOpType.add,
                        )
                        rowsrc = outT[:]
                    # transpose back to row-major [128, D] and store
                    trp = trpp.tile([P, D], f32, space="PSUM")
                    nc.tensor.transpose(
                        out=trp[:], in_=rowsrc, identity=ident_sb[:D, :D]
                    )
                    hrow = epi.tile([P, D], f32, tag="hrow")
                    nc.vector.tensor_copy(out=hrow[:], in_=trp[:])
                    nc.sync.dma_start(
                        out=dst_dram[t * P : (t + 1) * P, :], in_=hrow[:]
                    )

            layer(x, xT_sb, w1l_sb, w1r_sb, b1_sb, True, h_own)
            nc.gpsimd.collective_compute(
                "AllGather",
                mybir.AluOpType.bypass,
                replica_groups=[list(range(NC))],
                ins=[h_own[:]],
                outs=[h_full[:]],
            )
            layer(h_full, hT_sb, w2l_sb, w2r_sb, b2_sb, False, out)

    _split_multi_waits(nc)
    return nc


def kernel(x, edge_index, W1l, W1r, b1, W2l, W2r, b2):
    from concourse import bass_utils

    trace = bool(int(os.environ.get("KERNEL_TRACE", "0")))
    if trace:
        _install_ntff_shim()

    x = np.asarray(x, dtype=np.float32)
    nblk_t, src_a, rel_a, inv_a = _prep_edges(np.asarray(edge_index))

    xpad = np.zeros((NPAD, D), np.float32)
    xpad[:N] = x
    xT = np.ascontiguousarray(xpad.T)  # [D, NPAD]
    iota_h = np.tile(np.arange(P, dtype=np.float32), (P, 1))

    common = {
        "x": xpad,
        "iota": iota_h,
        "w1l": np.ascontiguousarray(np.asarray(W1l, np.float32).T),
        "w1r": np.ascontiguousarray(np.asarray(W1r, np.float32).T),
        "w2l": np.ascontiguousarray(np.asarray(W2l, np.float32).T),
        "w2r": np.ascontiguousarray(np.asarray(W2r, np.float32).T),
        "b1": np.asarray(b1, np.float32).reshape(D, 1),
        "b2": np.asarray(b2, np.float32).reshape(D, 1),
    }
    in_maps = []
    for c in range(NC):
        m = dict(common)
        m["xT"] = np.ascontiguousarray(xT[:, c * S : (c + 1) * S])
        m["srcs"] = np.ascontiguousarray(src_a[c])
        m["rels"] = np.ascontiguousarray(rel_a[c])
        m["invs"] = np.ascontiguousarray(inv_a[c])
        in_maps.append(m)

    nc = _build_nc(nblk_t)
    br = bass_utils.run_bass_kernel_spmd(
        nc, in_maps, core_ids=list(range(NC)), trace=trace
    )
    if trace:
        print(f"HW exec time: {br.exec_time_ns} ns")

    out = np.concatenate([br.results[c]["out"] for c in range(NC)], axis=0)
    return out[:N]
